# revision 12
# baseline (speedup 1.0000x reference)
"""DopplerPTNet point-transformer block on 8 Trainium2 NeuronCores.

v2: fp8 tables + restructured per-tile pipeline.

  - Shard N points across 8 cores; replicate small weights.
  - Per core, build a 520-byte kv row per point: [ k fp8e3 256B | v fp8e3
    256B | tg bf16 6B | pad 2B ], k = feats@Wk.T, v = feats@Wv.T (biases
    folded into downstream shifts), tg = A1@xyz + c1.  AllGather the table
    in 4 chunks (overlapped with the build); host remaps idx to the
    chunk-major AllGather layout.  A width-8 fat table of vh = velocity
    encodings is AllGathered too.
  - Main pass per 128-point tile: 16 indirect row gathers (point-major
    stag), one transposed SBUF dma_gather for k (pair-interleaved fp8
    channel-major), a width-8 fat gather + one-hot select for vh.
    Logits psum chunks [128, 512]: per-ns pe matmuls (block lhsT consts),
    -q ident, +k ident (strided fp8 views), ACT relu with per-partition
    shift (all BN/bias folding host-side; channel order = even|odd from
    the pair transpose).  W1 (rows permuted), W2 per ns, softmax, vals =
    pe_v (T2_perm stationary) + v ident (natural order), prod from PSUM,
    ns tree-reduce, residual + rho.
"""

import sys

sys.path.insert(0, "/opt/trn_rl_repo")

import numpy as np
import ml_dtypes

import concourse.bass as bass
import concourse.mybir as mybir
import concourse.tile as tile
from concourse import bacc
from concourse.bass import IndirectOffsetOnAxis
from concourse.bass_utils import run_bass_kernel_spmd
from concourse.masks import make_identity

BF16 = mybir.dt.bfloat16
F8 = mybir.dt.float8e3
F32 = mybir.dt.float32
I32 = mybir.dt.int32
I16 = mybir.dt.int16
AOP = mybir.AluOpType
AFT = mybir.ActivationFunctionType

NCORES = 8
C = 256
NS = 16
CS = 32
S = 8
P = 128
ROWB = 520         # bytes: k 256 | v 256 | tg f8 3 | pad 5
EPS = 1e-5
NE = NS * P        # 2048 edges per tile
NCHUNK = 4         # AllGather chunks

f8np = ml_dtypes.float8_e3m4
bfnp = ml_dtypes.bfloat16


def _bf(x):
    return np.ascontiguousarray(np.asarray(x, np.float32).astype(bfnp))


def _f32(x):
    return np.ascontiguousarray(x, dtype=np.float32)


def build_program(n_total: int, debug_taps: bool = False):
    npc = n_total // NCORES
    nt = npc // P
    ntc = nt // NCHUNK            # tiles per AG chunk
    rows_c = npc // NCHUNK        # rows per AG chunk
    nf8 = n_total // 8            # fat vh rows (8 points each)

    nc = bacc.Bacc(
        "TRN2",
        target_bir_lowering=False,
        debug=False,
        enable_asserts=False,
        num_devices=NCORES,
    )

    def inp(name, shape, dt):
        return nc.dram_tensor(name, shape, dt, kind="ExternalInput")

    feats_sh = inp("feats_sh", [npc, C], F32)
    xyz_sh = inp("xyz_sh", [npc, 3], F32)
    vel_sh = inp("vel_sh", [npc, 1], F32)
    idx_sh = inp("idx_sh", [npc, NS], I32)        # remapped to AG layout
    idxv8_sh = inp("idxv8_sh", [nt, P, NE // 16], I16)
    vsel8_sh = inp("vsel8_sh", [npc, NS], BF16)   # idx_v % 8

    wq_t = inp("wq_t", [2, P, C], BF16)       # Wq.T, out cols pi-ordered
    wkv_t = inp("wkv_t", [2, P, 2 * C], BF16)  # [Wk.T | Wv.T] natural
    w2w_ns = inp("w2w_ns", [64, 2 * NS * P], BF16)  # block lhsT consts
    w2v_ns = inp("w2v_ns", [64, NS * C], BF16)      # block rhs consts
    w1_t = inp("w1_t", [2, P, CS], BF16)      # (w_w1*scale1).T pi rows
    w2_t = inp("w2_t", [CS, CS], BF16)        # (w_w2*scale2).T
    rw2 = inp("rw2", [2, P, C], BF16)         # r_w.T
    a1_t = inp("a1_t", [4, 4], F32)
    sh1_c = inp("sh1_c", [P, 2], F32)         # shift1'' per pi position
    sh2_c = inp("sh2_c", [CS, 1], F32)        # shift2'
    svbv_c = inp("svbv_c", [1, 2], F32)
    c1_c = inp("c1_c", [1, 4], F32)
    scaler_c = inp("scaler_c", [1, C], BF16)
    shiftr_c = inp("shiftr_c", [1, C], BF16)
    rb_c = inp("rb_c", [1, C], F32)
    s2idx_c = inp("s2idx_c", [P, P], I16)
    viota8_c = inp("viota8_c", [1, 8], BF16)

    out_ext = nc.dram_tensor("out", [npc, C], F32, kind="ExternalOutput")
    taps = {}
    if debug_taps:
        for nm, shp, dt in [
            ("tap_stag", [P, NS * ROWB], F8),
            ("tap_kcm", [P, 2 * NE], F8),
            ("tap_q", [P, 2 * P], BF16),
            ("tap_T2", [64, P], BF16),
            ("tap_pmx", [P, NS * 4], BF16),
            ("tap_t1", [P, 2 * NE], BF16),
            ("tap_t2", [CS, NE], BF16),
            ("tap_attnE", [P, NS * CS], BF16),
            ("tap_attnN", [P, NS * CS], BF16),
            ("tap_velf", [P, NS * 64], F32),
            ("tap_agg", [P, C], BF16),
            ("tap_rin", [P, C], BF16),
        ]:
            taps[nm] = nc.dram_tensor(nm, shp, dt, kind="ExternalOutput")

    # internal DRAM
    featsb = nc.dram_tensor("featsb", [npc, C], BF16)
    kv_shard = [
        nc.dram_tensor(f"kv_shard{q}", [rows_c, ROWB], F8)
        for q in range(NCHUNK)
    ]
    kv_full = nc.dram_tensor("kv_full", [n_total, ROWB], F8,
                             addr_space="Shared")
    vh_shard = nc.dram_tensor("vh_shard", [npc, 1], F32)
    vh8_shard = nc.dram_tensor("vh8_shard", [npc // 8, 64], F32)
    vh8_full = nc.dram_tensor("vh8_full", [nf8, 64], F32,
                              addr_space="Shared")

    rg = [list(range(NCORES))]

    def tap(t, nm, ap):
        if debug_taps and t == 0:
            nc.sync.dma_start(out=taps[nm][:, :], in_=ap)

    with tile.TileContext(nc) as tc:
        with (
            tc.tile_pool(name="const", bufs=1) as cpool,
            tc.tile_pool(name="work", bufs=2) as pool,
            tc.tile_pool(name="big", bufs=3) as bigpool,
            tc.tile_pool(name="ps_log", bufs=2, space="PSUM") as ps_log,
            tc.tile_pool(name="ps_val", bufs=2, space="PSUM") as ps_val,
            tc.tile_pool(name="ps_sm", bufs=2, space="PSUM") as ps_sm,
            tc.tile_pool(name="ps_misc", bufs=2, space="PSUM") as ps_misc,
        ):
            # ---------- constants ----------
            ident_b = cpool.tile([P, P], BF16, tag="ident_b")
            make_identity(nc, ident_b[:])
            ident_f = cpool.tile([P, P], F32, tag="ident_f")
            make_identity(nc, ident_f[:])
            ident_8 = cpool.tile([P, P], F8, tag="ident_8")
            nc.vector.tensor_copy(ident_8[:], ident_b[:])
            nident_b = cpool.tile([P, P], BF16, tag="nident_b")
            nc.vector.tensor_scalar_mul(nident_b[:], ident_b[:], -1.0)

            def cload(src, shape, dt, tag):
                t = cpool.tile(shape, dt, tag=tag)
                nc.sync.dma_start(out=t[:], in_=src)
                return t

            def gload(src, width, tag):
                t = cpool.tile([P, 2, width], BF16, tag=tag)
                for g in range(2):
                    nc.sync.dma_start(out=t[:, g, :], in_=src[g, :, :])
                return t

            wq_sb = gload(wq_t, C, "wq")
            wkv_sb = gload(wkv_t, 2 * C, "wkv")
            w1_sb = gload(w1_t, CS, "w1")
            rw2_sb = gload(rw2, C, "rw2")
            w2w_sb = cload(w2w_ns[:, :], [64, 2 * NS * P], BF16, "w2wns")
            w2v_sb = cload(w2v_ns[:, :], [64, NS * C], BF16, "w2vns")
            w2_sb = cload(w2_t[:, :], [CS, CS], BF16, "w2")
            a1_sb = cload(a1_t[:, :], [4, 4], F32, "a1")
            sh1_sb = cload(sh1_c[:, :], [P, 2], F32, "sh1")
            sh2_sb = cload(sh2_c[:, :], [CS, 1], F32, "sh2")
            s2idx_sb = cload(s2idx_c[:, :], [P, P], I16, "s2i")

            def bcast(name, src, width, dt):
                row = cpool.tile([1, width], dt, tag=name + "r")
                nc.sync.dma_start(out=row[:], in_=src)
                full = cpool.tile([P, width], dt, tag=name)
                nc.gpsimd.partition_broadcast(full[:], row[:])
                return full

            svbv_sb = bcast("svbv", svbv_c[:, :], 2, F32)
            c1_sb = bcast("c1", c1_c[:, :], 4, F32)
            sclr_sb = bcast("sclr", scaler_c[:, :], C, BF16)
            shfr_sb = bcast("shfr", shiftr_c[:, :], C, BF16)
            rb_sb = bcast("rb", rb_c[:, :], C, F32)
            viota8_sb = bcast("viota8", viota8_c[:, :], 8, BF16)

            # ---------- phase A: feats -> bf16 ----------
            rows_per = min(npc, 1024)
            cast_cols = rows_per * C // P
            for ch in range(npc // rows_per):
                sl = slice(ch * rows_per, (ch + 1) * rows_per)
                cb = bigpool.tile([P, cast_cols], BF16, tag="castbuf")
                nc.gpsimd.dma_start(
                    out=cb[:],
                    in_=feats_sh[sl, :].rearrange("(p j) c -> p (j c)", p=P),
                )
                nc.sync.dma_start(
                    out=featsb[sl, :].rearrange("(p j) c -> p (j c)", p=P),
                    in_=cb[:],
                )

            # ---------- phase B0: vh table + AllGather (overlaps B) ----------
            vh_acc = cpool.tile([P, nt], F32, tag="vh_acc")
            for t in range(nt):
                rsl = slice(t * P, (t + 1) * P)
                vel_t = pool.tile([P, 1], F32, tag="vel_t")
                nc.sync.dma_start(out=vel_t[:], in_=vel_sh[rsl, :])
                nc.scalar.activation(
                    vh_acc[:, t:t + 1],
                    vel_t[:],
                    AFT.Relu,
                    bias=svbv_sb[:, 1:2],
                    scale=svbv_sb[:, 0:1],
                )
            vhT_ps = ps_misc.tile([P, P], F32, tag="pmisc")
            nc.tensor.transpose(
                out=vhT_ps[:nt, :], in_=vh_acc[:], identity=ident_f[:]
            )
            vhT_sb = pool.tile([P, P], F32, tag="vhT_sb")
            nc.vector.tensor_copy(vhT_sb[:nt, :], vhT_ps[:nt, :])
            nc.sync.dma_start(
                out=vh_shard[:, :].rearrange("(t p) o -> t (p o)", p=P),
                in_=vhT_sb[:nt, :],
            )
            zro = pool.tile([P, npc // 8 * 64 // P], F32, tag="zro")
            nc.vector.memset(zro[:], 0)
            nc.sync.dma_start(
                out=vh8_shard[:, :].rearrange("(p x) k -> p (x k)", p=P),
                in_=zro[:],
            )
            vhp = pool.tile([P, npc // P], F32, tag="vhp")
            nc.sync.dma_start(
                out=vhp[:],
                in_=vh_shard[:, :].rearrange("(p x) o -> p (x o)", p=P),
            )
            nc.sync.dma_start(
                out=vh8_shard[:, :]
                .rearrange("(p r) (j e) -> p r j e", p=P, j=8)[:, :, :, 0],
                in_=vhp[:, :].rearrange("p (r j) -> p r j", j=8),
            )
            nc.gpsimd.collective_compute(
                "AllGather",
                AOP.bypass,
                replica_groups=rg,
                ins=[vh8_shard.ap().opt()],
                outs=[vh8_full.ap().opt()],
            )

            # ---------- phase B: kv tables + chunked AllGather ----------
            for t in range(nt):
                rsl = slice(t * P, (t + 1) * P)
                q = t // ntc
                lsl = slice((t - q * ntc) * P, (t - q * ntc + 1) * P)
                frow = pool.tile([P, C], BF16, tag="frow")
                nc.sync.dma_start(out=frow[:], in_=featsb[rsl, :])
                ftT = pool.tile([P, 2, P], BF16, tag="ftT")
                for g in range(2):
                    fT_ps = ps_misc.tile([P, P], BF16, tag="pmisc")
                    nc.tensor.transpose(
                        out=fT_ps[:],
                        in_=frow[:, g * P:(g + 1) * P],
                        identity=ident_b[:],
                    )
                    nc.scalar.copy(ftT[:, g, :], fT_ps[:])
                kv_ps = ps_val.tile([P, 2 * C], F32, tag="pval")
                for g in range(2):
                    nc.tensor.matmul(
                        out=kv_ps[:],
                        lhsT=ftT[:, g, :],
                        rhs=wkv_sb[:, g, :],
                        start=(g == 0),
                        stop=(g == 1),
                    )
                row_t = pool.tile([P, ROWB], F8, tag="row_t")
                nc.scalar.copy(row_t[:, 0:2 * C], kv_ps[:])

                xyz_t = pool.tile([P, 3], F32, tag="xyz_t")
                nc.sync.dma_start(out=xyz_t[:], in_=xyz_sh[rsl, :])
                xT_ps = ps_misc.tile([P, P], F32, tag="pmisc")
                nc.tensor.transpose(
                    out=xT_ps[:3, :], in_=xyz_t[:], identity=ident_f[:]
                )
                xT_sb = pool.tile([4, P], F32, tag="xT_sb")
                nc.vector.tensor_copy(xT_sb[:3, :], xT_ps[:3, :])
                tg_ps = ps_misc.tile([P, P], F32, tag="pmisc")
                nc.tensor.matmul(
                    out=tg_ps[:, :3],
                    lhsT=xT_sb[:3, :],
                    rhs=a1_sb[:3, :3],
                    start=True,
                    stop=True,
                )
                nc.vector.scalar_tensor_tensor(
                    out=row_t[:, 2 * C:2 * C + 3],
                    in0=tg_ps[:, :3],
                    scalar=1.0,
                    in1=c1_sb[:, :3],
                    op0=AOP.mult,
                    op1=AOP.add,
                )
                nc.sync.dma_start(out=kv_shard[q][lsl, :], in_=row_t[:])
                if t % ntc == ntc - 1:
                    nc.gpsimd.collective_compute(
                        "AllGather",
                        AOP.bypass,
                        replica_groups=rg,
                        ins=[kv_shard[q].ap().opt()],
                        outs=[
                            kv_full[
                                q * NCORES * rows_c:(q + 1) * NCORES * rows_c,
                                :,
                            ].opt()
                        ],
                    )

            # ---------- phase D: main pass ----------
            for t in range(nt):
                rsl = slice(t * P, (t + 1) * P)

                idx_t = pool.tile([P, NS], I32, tag="idx_t")
                nc.sync.dma_start(out=idx_t[:], in_=idx_sh[rsl, :])

                # stage-1: 16 indirect row gathers, point-major stag
                stag = bigpool.tile([P, NS, ROWB], F8, tag="stag")
                for s in range(NS):
                    nc.gpsimd.indirect_dma_start(
                        out=stag[:, s, :],
                        out_offset=None,
                        in_=kv_full[:, :],
                        in_offset=IndirectOffsetOnAxis(
                            ap=idx_t[:, s:s + 1], axis=0
                        ),
                    )

                # velocity: width-8 fat gather + one-hot
                idxv8_t = pool.tile([P, NE // 16], I16, tag="idxv8_t")
                nc.sync.dma_start(out=idxv8_t[:], in_=idxv8_sh[t, :, :])
                velf = bigpool.tile([P, NS, 64], F32, tag="velf")
                nc.gpsimd.dma_gather(
                    out_ap=velf[:, :, :],
                    in_ap=vh8_full[:, :],
                    idxs_ap=idxv8_t[:, :],
                    num_idxs=NE,
                    num_idxs_reg=NE,
                    elem_size=64,
                    transpose=False,
                    single_packet=False,
                )
                vsel_t = pool.tile([P, NS], BF16, tag="vsel_t")
                nc.sync.dma_start(out=vsel_t[:], in_=vsel8_sh[rsl, :])
                vmask = pool.tile([P, NS, 8], BF16, tag="vmask")
                nc.vector.tensor_tensor(
                    out=vmask[:, :, :],
                    in0=viota8_sb[:, :]
                    .rearrange("p (o k) -> p o k", o=1)
                    .to_broadcast([P, NS, 8]),
                    in1=vsel_t[:, :]
                    .rearrange("p (s o) -> p s o", o=1)
                    .to_broadcast([P, NS, 8]),
                    op=AOP.is_equal,
                )
                vprod = pool.tile([P, NS, 8], F32, tag="vprod")
                nc.vector.tensor_tensor(
                    out=vprod[:, :, :],
                    in0=velf[:, :, :].rearrange(
                        "p s (j e) -> p s j e", j=8
                    )[:, :, :, 0],
                    in1=vmask[:, :, :],
                    op=AOP.mult,
                )

                # pmx [128, (ns,4)] bf16: relu(tg - axc), vh
                xyz_t = pool.tile([P, 3], F32, tag="xyz_t")
                nc.sync.dma_start(out=xyz_t[:], in_=xyz_sh[rsl, :])
                xT_ps = ps_misc.tile([P, P], F32, tag="pmisc")
                nc.tensor.transpose(
                    out=xT_ps[:3, :], in_=xyz_t[:], identity=ident_f[:]
                )
                xT_sb = pool.tile([4, P], F32, tag="xT_sb")
                nc.vector.tensor_copy(xT_sb[:3, :], xT_ps[:3, :])
                axc_ps = ps_misc.tile([P, P], F32, tag="pmisc")
                nc.tensor.matmul(
                    out=axc_ps[:, :3],
                    lhsT=xT_sb[:3, :],
                    rhs=a1_sb[:3, :3],
                    start=True,
                    stop=True,
                )
                pmx = pool.tile([P, NS, 4], BF16, tag="pmx")
                nc.vector.scalar_tensor_tensor(
                    out=pmx[:, :, 0:3],
                    in0=stag[:, :, 2 * C:2 * C + 3],
                    scalar=1.0,
                    in1=axc_ps[:, :3]
                    .rearrange("p (o d) -> p o d", o=1)
                    .to_broadcast([P, NS, 3]),
                    op0=AOP.mult,
                    op1=AOP.subtract,
                )
                nc.vector.tensor_scalar_max(pmx[:, :, 0:3], pmx[:, :, 0:3],
                                            0.0)
                # vh tree-select over 8 -> pmx[:, :, 3]
                vt1 = pool.tile([P, NS, 4], F32, tag="vt1")
                nc.vector.tensor_tensor(
                    out=vt1[:, :, :], in0=vprod[:, :, 0:8:2],
                    in1=vprod[:, :, 1:8:2], op=AOP.add)
                vt2 = pool.tile([P, NS, 2], F32, tag="vt2")
                nc.vector.tensor_tensor(
                    out=vt2[:, :, :], in0=vt1[:, :, 0:4:2],
                    in1=vt1[:, :, 1:4:2], op=AOP.add)
                nc.vector.tensor_tensor(
                    out=pmx[:, :, 3:4], in0=vt2[:, :, 0:1],
                    in1=vt2[:, :, 1:2], op=AOP.add)

                # T2_perm [64, 128] = pmx^T
                T2_ps = ps_misc.tile([P, P], BF16, tag="pmisc")
                nc.tensor.transpose(
                    out=T2_ps[:64, :],
                    in_=pmx[:, :, :].rearrange("p s d -> p (s d)"),
                    identity=ident_b[:],
                )
                T2p = pool.tile([64, P], BF16, tag="T2p")
                nc.vector.tensor_copy(T2p[:, :], T2_ps[:64, :])

                # feats tiles (PE transposes; avoids DMA-transpose vs
                # collective serialization and two HWDGE calls)
                feats_pm = pool.tile([P, C], BF16, tag="feats_pm")
                nc.sync.dma_start(out=feats_pm[:], in_=featsb[rsl, :])
                ftT = pool.tile([P, 2, P], BF16, tag="ftT")
                for g in range(2):
                    fT_ps = ps_misc.tile([P, P], BF16, tag="pmisc")
                    nc.tensor.transpose(
                        out=fT_ps[:],
                        in_=feats_pm[:, g * P:(g + 1) * P],
                        identity=ident_b[:],
                    )
                    nc.scalar.copy(ftT[:, g, :], fT_ps[:])

                # q channel-major (pi order), 4-ns replica
                q_sb = pool.tile([P, 2, P], BF16, tag="q_sb")
                q_rep = pool.tile([P, 2, 4 * P], BF16, tag="q_rep")
                for cg in range(2):
                    q_ps = ps_misc.tile([P, P], F32, tag="pmisc")
                    for g in range(2):
                        nc.tensor.matmul(
                            out=q_ps[:],
                            lhsT=wq_sb[:, g, cg * P:(cg + 1) * P],
                            rhs=ftT[:, g, :],
                            start=(g == 0),
                            stop=(g == 1),
                        )
                    nc.scalar.copy(q_sb[:, cg, :], q_ps[:])
                    nc.vector.tensor_copy(
                        q_rep[:, cg, :],
                        q_sb[:, cg, :]
                        .rearrange("p (o n) -> p o n", o=1)
                        .to_broadcast([P, 4, P]),
                    )

                # stage-2: k -> channel-major fp8 pairs
                k_cm = bigpool.tile([P, 2, NE], F8, tag="k_cm")
                nc.gpsimd.dma_gather(
                    out_ap=k_cm[:, :, :],
                    in_ap=stag[:, :, :].rearrange("p s r -> p (s r)"),
                    idxs_ap=s2idx_sb[:, :],
                    num_idxs=NE,
                    num_idxs_reg=NE,
                    elem_size=C,
                    transpose=True,
                    sbuf_tokens_per_rank=P,
                    sbuf_free_dim_per_rank=ROWB,
                    sbuf_free_dim_pad_per_rank=ROWB - 2 * C,
                    sbuf_byte_offset=0,
                    single_packet=False,
                )
                kpair = (
                    k_cm[:, :, :]
                    .rearrange("p two e -> p (two e)")
                    .rearrange("p (e two) -> p e two", two=2)
                )
                tap(t, "tap_stag", stag[:, :, :].rearrange("p s r -> p (s r)"))
                tap(t, "tap_kcm", k_cm[:, :, :].rearrange("p a e -> p (a e)"))
                tap(t, "tap_q", q_sb[:, :, :].rearrange("p g n -> p (g n)"))
                tap(t, "tap_T2", T2p[:, :])
                tap(t, "tap_pmx", pmx[:, :, :].rearrange("p s d -> p (s d)"))
                tap(t, "tap_velf", velf[:, :, :].rearrange("p s k -> p (s k)"))

                # logits: 8 psum chunks of [128, 512]
                t1 = bigpool.tile([P, 2, NE], BF16, tag="t1")
                for cg in range(2):
                    for g in range(4):
                        w_ps = ps_log.tile([P, 4 * P], F32, tag="plog")
                        nc.tensor.matmul(
                            out=w_ps[:],
                            lhsT=nident_b[:],
                            rhs=q_rep[:, cg, :],
                            start=True,
                            stop=False,
                        )
                        for j in range(4):
                            ns = g * 4 + j
                            nc.tensor.matmul(
                                out=w_ps[:, j * P:(j + 1) * P],
                                lhsT=w2w_sb[:, :].rearrange(
                                    "k (cg s p) -> k cg s p",
                                    cg=2, s=NS)[:, cg, ns, :],
                                rhs=T2p[:, :],
                                start=False,
                                stop=False,
                            )
                        nc.tensor.matmul(
                            out=w_ps[:],
                            lhsT=ident_8[:],
                            rhs=kpair[:, g * 4 * P:(g + 1) * 4 * P, cg],
                            start=False,
                            stop=True,
                        )
                        nc.scalar.activation(
                            t1[:, cg, g * 4 * P:(g + 1) * 4 * P],
                            w_ps[:],
                            AFT.Relu,
                            bias=sh1_sb[:, cg:cg + 1],
                            scale=1.0,
                        )

                tap(t, "tap_t1", t1[:, :, :].rearrange("p g n -> p (g n)"))
                # W1 (pi rows) + relu/shift2
                t2 = bigpool.tile([CS, NE], BF16, tag="t2")
                for g in range(4):
                    w1_ps = ps_sm.tile([CS, 4 * P], F32, tag="psm")
                    for cg in range(2):
                        nc.tensor.matmul(
                            out=w1_ps[:],
                            lhsT=w1_sb[:, cg, :],
                            rhs=t1[:, cg, g * 4 * P:(g + 1) * 4 * P],
                            start=(cg == 0),
                            stop=(cg == 1),
                        )
                    nc.scalar.activation(
                        t2[:, g * 4 * P:(g + 1) * 4 * P],
                        w1_ps[:],
                        AFT.Relu,
                        bias=sh2_sb[:, 0:1],
                        scale=1.0,
                    )

                tap(t, "tap_t2", t2[:, :])
                # W2 per-ns -> point-major logits, exp
                attn_ps = ps_sm.tile([P, NS * CS], F32, tag="psm")
                for ns in range(NS):
                    nc.tensor.matmul(
                        out=attn_ps[:, ns * CS:(ns + 1) * CS],
                        lhsT=t2[:, ns * P:(ns + 1) * P],
                        rhs=w2_sb[:, :],
                        start=True,
                        stop=True,
                    )
                attnE = pool.tile([P, NS * CS], BF16, tag="attnE")
                nc.scalar.activation(attnE[:], attn_ps[:], AFT.Exp)

                tap(t, "tap_attnE", attnE[:, :])
                # softmax denom: tree over ns
                scr = pool.tile([P, 12 * CS], BF16, tag="scr")
                v0 = attnE[:, :].rearrange("p (s c) -> p s c", c=CS)
                r1 = scr[:, 0:8 * CS].rearrange("p (s c) -> p s c", c=CS)
                nc.vector.tensor_tensor(
                    out=r1, in0=v0[:, 0:16:2, :], in1=v0[:, 1:16:2, :],
                    op=AOP.add)
                r2 = scr[:, 8 * CS:12 * CS].rearrange(
                    "p (s c) -> p s c", c=CS)
                nc.vector.tensor_tensor(
                    out=r2, in0=r1[:, 0:8:2, :], in1=r1[:, 1:8:2, :],
                    op=AOP.add)
                ssum = pool.tile([P, CS], F32, tag="ssum")
                s3 = pool.tile([P, 2 * CS], F32, tag="s3")
                s3v = s3[:, :].rearrange("p (s c) -> p s c", c=CS)
                nc.vector.tensor_tensor(
                    out=s3v, in0=r2[:, 0:4:2, :], in1=r2[:, 1:4:2, :],
                    op=AOP.add)
                nc.vector.tensor_tensor(
                    out=ssum[:].rearrange("p (s c) -> p s c", c=CS),
                    in0=s3v[:, 0:1, :], in1=s3v[:, 1:2, :], op=AOP.add)
                rcp = pool.tile([P, CS], F32, tag="rcp")
                nc.vector.reciprocal(rcp[:], ssum[:])
                attn_n = pool.tile([P, NS * CS], BF16, tag="attn_n")
                nc.vector.tensor_tensor(
                    out=attn_n[:].rearrange("p (s c) -> p s c", c=CS),
                    in0=attnE[:].rearrange("p (s c) -> p s c", c=CS),
                    in1=rcp[:]
                    .rearrange("p (o c) -> p o c", o=1)
                    .to_broadcast([P, NS, CS]),
                    op=AOP.mult,
                )

                tap(t, "tap_attnN", attn_n[:, :])
                # vals chunks: pe_v + v ident, prod from PSUM
                prod = bigpool.tile([P, NS, C], BF16, tag="prod")
                for qt in range(8):
                    v_ps = ps_val.tile([P, 2 * C], F32, tag="pval")
                    nc.tensor.matmul(
                        out=v_ps[:, :],
                        lhsT=T2p[:, :],
                        rhs=w2v_sb[:, :].rearrange(
                            "k (s c) -> k s c", s=NS)[:, 2 * qt:2 * qt + 2, :],
                        start=True,
                        stop=False,
                    )
                    nc.tensor.matmul(
                        out=v_ps[:, :],
                        lhsT=ident_8[:],
                        rhs=stag[:, 2 * qt:2 * qt + 2, C:2 * C],
                        start=False,
                        stop=True,
                    )
                    nc.vector.tensor_tensor(
                        out=prod[:, 2 * qt:2 * qt + 2, :].rearrange(
                            "p s (g c) -> p s g c", c=CS),
                        in0=v_ps[:, :].rearrange(
                            "p (s g c) -> p s g c", s=2, c=CS),
                        in1=attn_n[:, 2 * qt * CS:(2 * qt + 2) * CS]
                        .rearrange("p (s o c) -> p s o c", o=1, c=CS)
                        .to_broadcast([P, 2, S, CS]),
                        op=AOP.mult,
                    )

                # ns tree-reduce
                tscr = bigpool.tile([P, 14 * C], BF16, tag="tscr")
                pv = prod[:, :, :]
                u1 = tscr[:, 0:8 * C].rearrange("p (s c) -> p s c", c=C)
                nc.vector.tensor_tensor(
                    out=u1, in0=pv[:, 0:16:2, :], in1=pv[:, 1:16:2, :],
                    op=AOP.add)
                u2 = tscr[:, 8 * C:12 * C].rearrange(
                    "p (s c) -> p s c", c=C)
                nc.vector.tensor_tensor(
                    out=u2, in0=u1[:, 0:8:2, :], in1=u1[:, 1:8:2, :],
                    op=AOP.add)
                u3 = tscr[:, 12 * C:14 * C].rearrange(
                    "p (s c) -> p s c", c=C)
                nc.vector.tensor_tensor(
                    out=u3, in0=u2[:, 0:4:2, :], in1=u2[:, 1:4:2, :],
                    op=AOP.add)
                agg = pool.tile([P, C], BF16, tag="agg")
                nc.vector.tensor_tensor(
                    out=agg[:].rearrange("p (s c) -> p s c", c=C),
                    in0=u3[:, 0:1, :], in1=u3[:, 1:2, :], op=AOP.add)

                tap(t, "tap_agg", agg[:, :])
                # residual + rho
                rin = pool.tile([P, C], BF16, tag="rin")
                nc.vector.tensor_tensor(
                    out=rin[:], in0=agg[:], in1=feats_pm[:], op=AOP.add)
                nc.vector.tensor_tensor(
                    out=rin[:], in0=rin[:], in1=sclr_sb[:], op=AOP.mult)
                nc.vector.tensor_tensor(
                    out=rin[:], in0=rin[:], in1=shfr_sb[:], op=AOP.add)
                nc.vector.tensor_scalar_max(rin[:], rin[:], 0.0)

                tap(t, "tap_rin", rin[:, :])
                rT_sb = pool.tile([P, 2, P], BF16, tag="rT_sb")
                for cg in range(2):
                    rT_ps = ps_misc.tile([P, P], BF16, tag="pmisc")
                    nc.tensor.transpose(
                        out=rT_ps[:],
                        in_=rin[:, cg * P:(cg + 1) * P],
                        identity=ident_b[:],
                    )
                    nc.vector.tensor_copy(rT_sb[:, cg, :], rT_ps[:])
                o_ps = ps_misc.tile([P, C], F32, tag="pmisc")
                for cg in range(2):
                    nc.tensor.matmul(
                        out=o_ps[:],
                        lhsT=rT_sb[:, cg, :],
                        rhs=rw2_sb[:, cg, :],
                        start=(cg == 0),
                        stop=(cg == 1),
                    )
                out_sb = pool.tile([P, C], F32, tag="out_sb")
                nc.vector.scalar_tensor_tensor(
                    out=out_sb[:],
                    in0=o_ps[:],
                    scalar=1.0,
                    in1=rb_sb[:],
                    op0=AOP.mult,
                    op1=AOP.add,
                )
                nc.sync.dma_start(out=out_ext[rsl, :], in_=out_sb[:])

    nc.compile()
    return nc


def prep_weights(inputs):
    """Host-side BN/bias folding, pi permutation, block consts."""
    # pi: psum position q (cg-major) -> channel
    pi = np.empty(C, np.int64)
    pi[0:P] = 2 * np.arange(P)          # cg0 = even channels
    pi[P:2 * P] = 2 * np.arange(P) + 1  # cg1 = odd channels

    g1, b1, m1, v1 = [np.asarray(inputs["w_bn1"][i], np.float64)
                      for i in range(4)]
    scale1 = g1 / np.sqrt(v1 + EPS)
    shift1 = (b1 / scale1 - m1
              + np.asarray(inputs["bk"], np.float64)
              - np.asarray(inputs["bq"], np.float64)
              + np.asarray(inputs["p_b2"], np.float64)
              + np.asarray(inputs["v_b2"], np.float64))

    wq = np.asarray(inputs["Wq"]).T     # [cin, cout]
    wq_pi = wq[:, pi]                   # out cols in pi order
    wq_t = np.stack([_bf(wq_pi[0:P]), _bf(wq_pi[P:2 * P])])
    wk = np.asarray(inputs["Wk"]).T
    wv = np.asarray(inputs["Wv"]).T
    wkv = np.concatenate([wk, wv], axis=1)
    wkv_t = np.stack([_bf(wkv[0:P]), _bf(wkv[P:2 * P])])

    gp, bp, mp, vp = [inputs["p_bn"][i] for i in range(4)]
    scale_p = gp / np.sqrt(vp + EPS)
    A1 = scale_p[:, None] * inputs["p_w1"]
    c1 = bp - scale_p * (mp - inputs["p_b1"])
    a1_t = np.zeros((4, 4), np.float32)
    a1_t[:3, :3] = A1.T

    gv, bv_, mv, vv = [inputs["v_bn"][i] for i in range(4)]
    scale_v = (gv / np.sqrt(vv + EPS))[0]
    sv = scale_v * inputs["v_w1"][0, 0]
    bvp = scale_v * (inputs["v_b1"][0] - mv[0]) + bv_[0]

    w2cat = np.zeros((4, C), np.float32)
    w2cat[0:3] = np.asarray(inputs["p_w2"]).T
    w2cat[3] = np.asarray(inputs["v_w2"])[:, 0]
    # block lhsT consts [64, 2, NS, 128]: rows (ns', d), nonzero at ns'==ns
    w2w_ns = np.zeros((64, 2, NS, P), np.float32)
    for ns in range(NS):
        for d in range(4):
            w2w_ns[4 * ns + d, 0, ns, :] = w2cat[d, pi[0:P]]
            w2w_ns[4 * ns + d, 1, ns, :] = w2cat[d, pi[P:2 * P]]
    # block rhs consts [64, NS, 256] natural channel order
    w2v_ns = np.zeros((64, NS, C), np.float32)
    for ns in range(NS):
        for d in range(4):
            w2v_ns[4 * ns + d, ns, :] = w2cat[d, :]

    g2, b2, m2, v2 = [np.asarray(inputs["w_bn2"][i], np.float64)
                      for i in range(4)]
    scale2 = g2 / np.sqrt(v2 + EPS)
    shift2 = b2 / scale2 - m2 + np.asarray(inputs["w_b1"], np.float64)
    w1s = (np.asarray(inputs["w_w1"], np.float64)
           * scale1[None, :]).T        # [256, 32]
    w1_pi = w1s[pi]
    w1_t = np.stack([_bf(w1_pi[0:P]), _bf(w1_pi[P:2 * P])])
    w2s = (np.asarray(inputs["w_w2"], np.float64) * scale2[None, :]).T
    w2_t = _bf(w2s)

    gr, br, mr, vr = [inputs["r_bn"][i] for i in range(4)]
    scale_r = gr / np.sqrt(vr + EPS)
    mean_r = mr - (inputs["bv"] + inputs["p_b2"] + inputs["v_b2"])
    shift_r = br - scale_r * mean_r
    rw2s = np.asarray(inputs["r_w"]).T
    rw2 = np.stack([_bf(rw2s[0:P]), _bf(rw2s[P:2 * P])])

    ii = np.arange(NE, dtype=np.int16)
    wrap = ii.reshape(P, 16).T
    s2idx = np.ascontiguousarray(np.tile(wrap, (8, 1)))

    sh1_pi = np.asarray(shift1, np.float64)[pi].astype(np.float32)

    return {
        "wq_t": wq_t,
        "wkv_t": wkv_t,
        "w2w_ns": _bf(w2w_ns.reshape(64, 2 * NS * P)),
        "w2v_ns": _bf(w2v_ns.reshape(64, NS * C)),
        "w1_t": w1_t,
        "w2_t": w2_t,
        "rw2": rw2,
        "a1_t": _f32(a1_t),
        "sh1_c": _f32(sh1_pi.reshape(2, P).T),
        "sh2_c": _f32(np.asarray(shift2, np.float32)[:, None]),
        "svbv_c": _f32(np.array([[sv, bvp]])),
        "c1_c": _f32(np.pad(np.asarray(c1, np.float64), (0, 1))[None, :]),
        "scaler_c": _bf(scale_r[None, :]),
        "shiftr_c": _bf(shift_r[None, :]),
        "rb_c": _f32(np.asarray(inputs["r_b"])[None, :]),
        "s2idx_c": s2idx,
        "viota8_c": _bf(np.arange(8, dtype=np.float32)[None, :]),
    }


def remap_idx(idx, n_total):
    """Map global point index -> row in the chunk-major AllGather layout."""
    npc = n_total // NCORES
    rows_c = npc // NCHUNK
    c = idx // npc
    r = idx % npc
    q = r // rows_c
    rp = r % rows_c
    return (q * (NCORES * rows_c) + c * rows_c + rp).astype(np.int32)


def remap_idx_v8(idx_v, n_total):
    """Map idx_v -> fat-8 row in the rank-major vh8 AllGather layout."""
    npc = n_total // NCORES
    c = idx_v // npc
    r = idx_v % npc
    return (c * (npc // 8) + r // 8).astype(np.int64)


def wrap_fat_idx8(fat):
    """Per-tile wrapped int16 layout for dma_gather."""
    npc = fat.shape[0]
    nt = npc // P
    fat = fat.astype(np.int16)
    out = np.empty((nt, P, NE // 16), np.int16)
    for t in range(nt):
        flat = fat[t * P:(t + 1) * P].T.ravel()  # i = ns*128 + n
        wrap = flat.reshape(NE // 16, 16).T
        out[t] = np.tile(wrap, (8, 1))
    return np.ascontiguousarray(out)


_PROGRAM_CACHE = {}


def shard_inputs(inputs, wd, c, npc):
    n_total = npc * NCORES
    sl = slice(c * npc, (c + 1) * npc)
    idx_re = remap_idx(np.asarray(inputs["idx"][sl], np.int64), n_total)
    idxv = np.asarray(inputs["idx_v"][sl], np.int64)
    fat8 = remap_idx_v8(idxv, n_total)
    m = {
        "feats_sh": _f32(inputs["feats"][sl]),
        "xyz_sh": _f32(inputs["xyz"][sl]),
        "vel_sh": _f32(inputs["velocities"][sl]),
        "idx_sh": np.ascontiguousarray(idx_re),
        "idxv8_sh": wrap_fat_idx8(fat8),
        "vsel8_sh": _bf(idxv % 8),
    }
    m.update(wd)
    return m


def run(inputs, n_total, debug_taps=False, **spmd_kwargs):
    npc = n_total // NCORES
    key = (n_total, debug_taps)
    if key not in _PROGRAM_CACHE:
        _PROGRAM_CACHE[key] = build_program(n_total, debug_taps)
    nc = _PROGRAM_CACHE[key]

    wd = prep_weights(inputs)
    in_maps = [shard_inputs(inputs, wd, c, npc) for c in range(NCORES)]

    res = run_bass_kernel_spmd(
        nc, in_maps, core_ids=list(range(NCORES)), **spmd_kwargs
    )
    out = np.concatenate([r["out"] for r in res.results], axis=0)
    return out, res


def kernel(**inputs):
    inputs = {k: np.asarray(v) for k, v in inputs.items()}
    n_total = inputs["feats"].shape[0]
    out, _ = run(inputs, n_total)
    return np.ascontiguousarray(out, dtype=np.float32)
